# revision 7
# baseline (speedup 1.0000x reference)
"""BolT windowed-attention kernel for 8 TRN2 NeuronCores (self-contained)."""
import sys, os
sys.path.insert(0, "/opt/trn_rl_repo")
import numpy as np
import ml_dtypes
from contextlib import ExitStack

from concourse import bass, bacc, tile
import concourse.mybir as mybir
from concourse import bass_utils
from concourse._compat import with_exitstack

bf16 = ml_dtypes.bfloat16

# Problem constants (hardcoded per spec)
NUM_HEADS = 36
HEAD_DIM = 20
B, NW_CONST = 32, 100
B_ = B * NW_CONST          # 3200
C = 400
N_TOK, M_TOK = 20, 40
Np1, Mp1 = 21, 41
NCORES = int(os.environ.get("BOLT_NCORES", "8"))
NWIN = int(os.environ.get("BOLT_NWIN", str(B_ // 8)))  # windows per core
WBLK = 8
NBLK = NWIN // WBLK
assert NWIN % WBLK == 0
KC = 512                   # padded contraction (400 C + 1 ones + pad)
NCHUNK = 4
NG = 9                     # 4-head groups
MASK_BIG = -30000.0
SIM = os.environ.get("BOLT_SIM", "0") == "1"

_cache = {}


def _ecopy(nc, use_vec, out, in_):
    if use_vec:
        nc.vector.tensor_copy(out, in_)
    else:
        nc.scalar.copy(out, in_)


def _build_program():
    nc = bacc.Bacc("TRN2", target_bir_lowering=False, debug=False,
                   num_devices=NCORES)
    dt = mybir.dt
    d_xq = nc.dram_tensor("xq", (NCHUNK, 128, NWIN * 32), dt.bfloat16,
                          kind="ExternalInput").ap()
    d_xk = nc.dram_tensor("xk", (NCHUNK, 128, NWIN * Mp1), dt.bfloat16,
                          kind="ExternalInput").ap()
    d_wq = nc.dram_tensor("wq", (NCHUNK, NG, 128, 128), dt.bfloat16,
                          kind="ExternalInput").ap()
    d_wk = nc.dram_tensor("wk", (NCHUNK, NG, 128, 128), dt.bfloat16,
                          kind="ExternalInput").ap()
    d_wv = nc.dram_tensor("wv", (NCHUNK, NG, 128, 128), dt.bfloat16,
                          kind="ExternalInput").ap()
    d_wp = nc.dram_tensor("wp", (NG, 4, 128, 128), dt.bfloat16,
                          kind="ExternalInput").ap()
    d_bias = nc.dram_tensor("biast", (NG, 128, Mp1), dt.float32,
                            kind="ExternalInput").ap()
    d_mask = nc.dram_tensor("maskt", (4, NG, 128, Mp1), dt.float32,
                            kind="ExternalInput").ap()
    d_bp = nc.dram_tensor("bp", (4, 128, 1), dt.float32,
                          kind="ExternalInput").ap()
    d_id = nc.dram_tensor("ident", (128, 128), dt.bfloat16,
                          kind="ExternalInput").ap()
    d_out = nc.dram_tensor("outT", (4, 128, NWIN * Np1), dt.float32,
                           kind="ExternalOutput").ap()

    @with_exitstack
    def kern(ctx: ExitStack, tc: tile.TileContext):
        nc = tc.nc
        const = ctx.enter_context(tc.tile_pool(name="const", bufs=1))
        xpool = ctx.enter_context(tc.tile_pool(name="x", bufs=2))
        fmpool = ctx.enter_context(tc.tile_pool(name="fm", bufs=2))
        atpool = ctx.enter_context(tc.tile_pool(name="at", bufs=1))
        smpool = ctx.enter_context(tc.tile_pool(name="sm", bufs=2))
        opool = ctx.enter_context(tc.tile_pool(name="o", bufs=2))
        ps_p = ctx.enter_context(tc.tile_pool(name="psp", bufs=2, space="PSUM"))
        ps_at = ctx.enter_context(tc.tile_pool(name="psat", bufs=2, space="PSUM"))
        ps_t = ctx.enter_context(tc.tile_pool(name="pst", bufs=1, space="PSUM"))
        ps_av = ctx.enter_context(tc.tile_pool(name="psav", bufs=2, space="PSUM"))

        t_wq = [[const.tile([128, 128], dt.bfloat16, tag=f"wq{c}_{a}", name=f"wq{c}_{a}")
                 for a in range(NG)] for c in range(NCHUNK)]
        t_wk = [[const.tile([128, 128], dt.bfloat16, tag=f"wk{c}_{a}", name=f"wk{c}_{a}")
                 for a in range(NG)] for c in range(NCHUNK)]
        t_wv = [[const.tile([128, 128], dt.bfloat16, tag=f"wv{c}_{a}", name=f"wv{c}_{a}")
                 for a in range(NG)] for c in range(NCHUNK)]
        t_wp = [[const.tile([128, 128], dt.bfloat16, tag=f"wp{a}_{m}", name=f"wp{a}_{m}")
                 for m in range(4)] for a in range(NG)]
        for c in range(NCHUNK):
            for a in range(NG):
                nc.sync.dma_start(t_wq[c][a][:], d_wq[c, a])
                nc.sync.dma_start(t_wk[c][a][:], d_wk[c, a])
                nc.sync.dma_start(t_wv[c][a][:], d_wv[c, a])
        for a in range(NG):
            for m in range(4):
                nc.sync.dma_start(t_wp[a][m][:], d_wp[a, m])
        t_bias = [const.tile([128, Mp1], dt.float32, tag=f"bias{a}", name=f"bias{a}")
                  for a in range(NG)]
        for a in range(NG):
            nc.sync.dma_start(t_bias[a][:], d_bias[a])
        t_mask = [[const.tile([128, Mp1], dt.float32, tag=f"mask{v}_{a}", name=f"mask{v}_{a}")
                   for a in range(NG)] for v in range(4)]
        for v in range(4):
            for a in range(NG):
                nc.sync.dma_start(t_mask[v][a][:], d_mask[v, a])
        t_bp = [const.tile([128, 1], dt.float32, tag=f"bp{m}", name=f"bpt{m}") for m in range(4)]
        for m in range(4):
            nc.sync.dma_start(t_bp[m][:], d_bp[m])
        t_id = const.tile([128, 128], dt.bfloat16, tag="ident", name="identt")
        nc.sync.dma_start(t_id[:], d_id[:])

        def mask_variant(wl):
            r = wl % NW_CONST
            if r == 0: return 0
            if r == 1: return 1
            if r == NW_CONST - 2: return 2
            if r == NW_CONST - 1: return 3
            return None

        for b in range(NBLK):
            w0 = b * WBLK
            W = WBLK
            NQ, NK = W * Np1, W * Mp1
            NQ2 = W * 32
            t_xq = [xpool.tile([128, NQ2], dt.bfloat16, tag=f"xq{c}", name=f"txq{b}_{c}")
                    for c in range(NCHUNK)]
            t_xk = [xpool.tile([128, NK], dt.bfloat16, tag=f"xk{c}", name=f"txk{b}_{c}")
                    for c in range(NCHUNK)]
            for c in range(NCHUNK):
                nc.sync.dma_start(t_xq[c][:], d_xq[c, :, w0*32:w0*32+NQ2])
                nc.sync.dma_start(t_xk[c][:], d_xk[c, :, w0*Mp1:w0*Mp1+NK])

            # --- projections ---
            t_q, t_k, t_v = [], [], []
            for kind, wmat, xin, nfree, dest in (
                ("q", t_wq, t_xq, NQ2, t_q),
                ("k", t_wk, t_xk, NK, t_k),
                ("v", t_wv, t_xk, NK, t_v),
            ):
                for a in range(NG):
                    pp = ps_p.tile([128, 512], dt.float32, tag="pp", name=f"pp{b}_{kind}{a}")[:, :nfree]
                    for c in range(NCHUNK):
                        nc.tensor.matmul(pp[:], wmat[c][a][:], xin[c][:],
                                         start=(c == 0), stop=(c == NCHUNK - 1))
                    sq = fmpool.tile([128, nfree], dt.bfloat16, tag=f"{kind}{a}", name=f"t{kind}{b}_{a}")
                    _ecopy(nc, a % 2 == 0, sq[:], pp[:])
                    dest.append(sq)

            # --- attention softmax per group ---
            t_n = []
            for a in range(NG):
                pat = ps_at.tile([128, 512], dt.float32, tag="pat", name=f"pat{b}_{a}")[:, :NK]
                for w in range(W):
                    for j in range(4):
                        nc.tensor.matmul(
                            pat[32*j:32*j+32, w*Mp1:(w+1)*Mp1],
                            t_q[a][32*j:32*j+32, w*32:(w+1)*32],
                            t_k[a][32*j:32*j+32, w*Mp1:(w+1)*Mp1],
                            start=True, stop=True,
                            tile_position=(32*j, 32*j),
                            skip_group_check=True,
                        )
                bb = t_bias[a][:].unsqueeze(1).broadcast_to((128, W, Mp1))
                pat3 = pat[:].rearrange("p (w m) -> p w m", w=W)
                nc.vector.tensor_add(pat3, pat3, bb)
                for w in range(W):
                    v = mask_variant(w0 + w)
                    if v is not None:
                        sl = pat[:, w*Mp1:(w+1)*Mp1]
                        nc.vector.tensor_add(sl, sl, t_mask[v][a][:])
                te = smpool.tile([128, NK], dt.bfloat16, tag="te", name=f"te{b}_{a}")
                nc.scalar.activation(te[:], pat[:],
                                     mybir.ActivationFunctionType.Exp)
                ts = smpool.tile([128, W], dt.float32, tag="ts", name=f"ts{b}_{a}")
                nc.vector.tensor_reduce(
                    ts[:], te[:].rearrange("p (w m) -> p w m", w=W),
                    axis=mybir.AxisListType.X, op=mybir.AluOpType.add)
                tr = smpool.tile([128, W], dt.float32, tag="tr", name=f"tr{b}_{a}")
                nc.vector.reciprocal(tr[:], ts[:])
                tn = smpool.tile([128, NK], dt.bfloat16, tag=f"tn{a}", name=f"tn{b}_{a}")
                rb = tr[:].unsqueeze(-1).broadcast_to((128, W, Mp1))
                nc.vector.tensor_mul(
                    tn[:].rearrange("p (w m) -> p w m", w=W),
                    te[:].rearrange("p (w m) -> p w m", w=W), rb)
                t_n.append(tn)

            # --- transposes: attn^T and v-nat per window ---
            t_aT, t_vT = [], []
            for w in range(W):
                taT = atpool.tile([Mp1, NG * 128], dt.bfloat16, tag=f"aT{w}", name=f"taT{b}_{w}")
                tvT = atpool.tile([Mp1, NG * 128], dt.bfloat16, tag=f"vT{w}", name=f"tvT{b}_{w}")
                for g0 in range(0, NG, 4):
                    gn = min(4, NG - g0)
                    paT = ps_t.tile([Mp1, 1024], dt.bfloat16, tag="paT", name=f"paT{b}_{w}_{g0}")[:, :512]
                    pvT = ps_t.tile([Mp1, 1024], dt.bfloat16, tag="pvT", name=f"pvT{b}_{w}_{g0}")[:, :512]
                    for gi in range(gn):
                        a = g0 + gi
                        nc.tensor.transpose(
                            paT[:, gi*128:(gi+1)*128],
                            t_n[a][:, w*Mp1:(w+1)*Mp1], t_id[:])
                        nc.tensor.transpose(
                            pvT[:, gi*128:(gi+1)*128],
                            t_v[a][:, w*Mp1:(w+1)*Mp1], t_id[:])
                    _ecopy(nc, w % 2 == 0, taT[:, g0*128:(g0+gn)*128],
                           paT[:, :gn*128])
                    _ecopy(nc, w % 2 == 1, tvT[:, g0*128:(g0+gn)*128],
                           pvT[:, :gn*128])
                t_aT.append(taT)
                t_vT.append(tvT)

            # --- AV ---
            t_ao = []
            for a in range(NG):
                pav = ps_av.tile([128, 512], dt.float32, tag="pav", name=f"pav{b}_{a}")[:, :NQ]
                for w in range(W):
                    for j in range(4):
                        cb = a * 128 + 32 * j
                        nc.tensor.matmul(
                            pav[32*j:32*j+32, w*Np1:(w+1)*Np1],
                            t_vT[w][:, cb:cb+32],
                            t_aT[w][:, cb:cb+21],
                            start=True, stop=True,
                            tile_position=(0, 32*j),
                            skip_group_check=True,
                        )
                so = fmpool.tile([128, NQ], dt.bfloat16, tag=f"ao{a}", name=f"tao{b}_{a}")
                _ecopy(nc, a % 2 == 0, so[:], pav[:])
                t_ao.append(so)

            # --- output projection ---
            for m in range(4):
                po = ps_p.tile([128, 512], dt.float32, tag="pp", name=f"po{b}_{m}")[:, :NQ]
                for a in range(NG):
                    nc.tensor.matmul(po[:], t_wp[a][m][:], t_ao[a][:],
                                     start=(a == 0), stop=(a == NG - 1))
                to = opool.tile([128, NQ], dt.float32, tag=f"to{m}", name=f"tto{b}_{m}")
                nc.vector.tensor_scalar(to[:], po[:], t_bp[m][:], None,
                                        mybir.AluOpType.add)
                nc.sync.dma_start(d_out[m, :, w0*Np1:w0*Np1+NQ], to[:])

    with tile.TileContext(nc) as tc:
        kern(tc)
    nc.compile()
    return nc


def _prep_host(x, x_, mask_left, mask_right, nW, Wq, bq, Wkv, bkv, Wproj,
               bproj, rel_table, cls_up, cls_down, cls_self):
    H, D = NUM_HEADS, HEAD_DIM
    scale = float(D) ** -0.5
    x = np.asarray(x); x_ = np.asarray(x_)
    Wq = np.asarray(Wq); bq = np.asarray(bq)
    Wkv = np.asarray(Wkv); bkv = np.asarray(bkv)
    Wproj = np.asarray(Wproj); bproj = np.asarray(bproj)
    rel_table = np.asarray(rel_table)
    cls_up = np.asarray(cls_up); cls_down = np.asarray(cls_down)
    cls_self = np.asarray(cls_self)

    maxDisparity = N_TOK - 1 + (M_TOK - N_TOK) // 2
    n_i = np.arange(N_TOK)[:, None]
    m_i = np.arange(M_TOK)[None, :]
    rel_idx = (n_i - (m_i - (M_TOK - N_TOK) // 2)) + maxDisparity
    rel_bias = np.transpose(rel_table[rel_idx], (2, 0, 1))
    top = np.concatenate([cls_self, cls_up], axis=-1)
    bot = np.concatenate([cls_down, rel_bias[None]], axis=-1)
    bias = np.concatenate([top, bot], axis=2)[0]              # (H,Np1,Mp1)

    def p32_weights(Wmat, bvec, scale_):
        out = np.zeros((NCHUNK, NG, 128, 128), dtype=np.float32)
        Wext = np.zeros((KC, 720), dtype=np.float32)
        Wext[:C] = Wmat.T * scale_
        Wext[C] = bvec * scale_
        for a in range(NG):
            for j in range(4):
                h = 4 * a + j
                blk = Wext[:, h*20:(h+1)*20]
                for c in range(NCHUNK):
                    out[c, a, :, 32*j:32*j+20] = blk[c*128:(c+1)*128]
        return out.astype(bf16)

    wq = p32_weights(Wq, bq, scale)
    wk = p32_weights(Wkv[:720], bkv[:720], 1.0)
    wv = p32_weights(Wkv[720:], bkv[720:], 1.0)

    wp = np.zeros((NG, 4, 128, 128), dtype=np.float32)
    for a in range(NG):
        for j in range(4):
            h = 4 * a + j
            for m in range(4):
                wp[a, m, 32*j:32*j+20, :100] = \
                    Wproj[m*100:(m+1)*100, h*20:(h+1)*20].T
    wp = wp.astype(bf16)

    biast = np.zeros((NG, 128, Mp1), dtype=np.float32)
    for a in range(NG):
        for j in range(4):
            biast[a, 32*j:32*j+21] = bias[4*a+j]
    mask_l = np.asarray(mask_left); mask_r = np.asarray(mask_right)
    maskt = np.zeros((4, NG, 128, Mp1), dtype=np.float32)
    for v in range(4):
        msk = mask_l[v] if v < 2 else mask_r[v - 2]
        add = np.where(msk == 1, MASK_BIG, 0.0).astype(np.float32)
        for a in range(NG):
            for j in range(4):
                maskt[v, a, 32*j:32*j+21] = add

    bp = np.zeros((4, 128, 1), dtype=np.float32)
    for m in range(4):
        bp[m, :100, 0] = bproj[m*100:(m+1)*100]
    ident = np.eye(128, dtype=np.float32).astype(bf16)

    xqs, xks = [], []
    for s in range(NCORES):
        xs = x[s*NWIN:(s+1)*NWIN].astype(np.float32)
        x_s = x_[s*NWIN:(s+1)*NWIN].astype(np.float32)
        xe = np.zeros((KC, NWIN, 32), dtype=np.float32)
        xe[:C, :, :Np1] = xs.transpose(2, 0, 1)
        xe[C, :, :Np1] = 1.0
        xe = xe.reshape(KC, NWIN * 32)
        xke = np.zeros((KC, NWIN * Mp1), dtype=np.float32)
        xke[:C] = x_s.reshape(-1, C).T
        xke[C] = 1.0
        xqs.append(np.ascontiguousarray(xe.reshape(NCHUNK, 128, -1)).astype(bf16))
        xks.append(np.ascontiguousarray(xke.reshape(NCHUNK, 128, -1)).astype(bf16))

    shared = dict(wq=wq, wk=wk, wv=wv, wp=wp, biast=biast, maskt=maskt,
                  bp=bp, ident=ident)
    return shared, xqs, xks


def kernel(**inputs) -> np.ndarray:
    if "prog" not in _cache:
        _cache["prog"] = _build_program()
    nc = _cache["prog"]
    shared, xqs, xks = _prep_host(**inputs)
    in_maps = []
    for s in range(NCORES):
        m = dict(shared)
        m["xq"] = xqs[s]
        m["xk"] = xks[s]
        in_maps.append(m)
    if SIM:
        from concourse.bass_interp import CoreSim
        outs = []
        for s in range(NCORES):
            sim = CoreSim(nc, trace=False, require_finite=False,
                          require_nnan=False)
            for k, v in in_maps[s].items():
                sim.tensor(k)[:] = v
            sim.simulate()
            outs.append(np.array(sim.tensor("outT")))
    else:
        res = bass_utils.run_bass_kernel_spmd(nc, in_maps,
                                              core_ids=list(range(NCORES)))
        outs = [res.results[s]["outT"] for s in range(NCORES)]
    pieces = []
    for s in range(NCORES):
        oT = outs[s]
        o = np.concatenate([oT[m, :100] for m in range(4)], axis=0)
        pieces.append(o.T.reshape(NWIN, Np1, C))
    return np.concatenate(pieces, axis=0).astype(np.float32)


# revision 12
# speedup vs baseline: 11.6209x; 11.6209x over previous
"""BolT windowed-attention kernel for 8 TRN2 NeuronCores (self-contained)."""
import sys, os
sys.path.insert(0, "/opt/trn_rl_repo")
import numpy as np
import ml_dtypes
from contextlib import ExitStack

from concourse import bass, bacc, tile
import concourse.mybir as mybir
from concourse import bass_utils
from concourse._compat import with_exitstack

bf16 = ml_dtypes.bfloat16

# Problem constants (hardcoded per spec)
NUM_HEADS = 36
HEAD_DIM = 20
B, NW_CONST = 32, 100
B_ = B * NW_CONST          # 3200
C = 400
N_TOK, M_TOK = 20, 40
Np1, Mp1 = 21, 41
NCORES = int(os.environ.get("BOLT_NCORES", "8"))
NWIN = int(os.environ.get("BOLT_NWIN", str(B_ // 8)))  # windows per core
WBLK = 8
NBLK = NWIN // WBLK
assert NWIN % WBLK == 0
KC = 512                   # padded contraction (400 C + 1 ones + pad)
NCHUNK = 4
NG = 9                     # 4-head groups
MASK_BIG = -30000.0
SIM = os.environ.get("BOLT_SIM", "0") == "1"

_cache = {}


def _ecopy(nc, use_vec, out, in_):
    if use_vec:
        nc.vector.tensor_copy(out, in_)
    else:
        nc.scalar.copy(out, in_)


def _build_program():
    nc = bacc.Bacc("TRN2", target_bir_lowering=False, debug=False,
                   num_devices=NCORES)
    dt = mybir.dt
    d_xq = nc.dram_tensor("xq", (NCHUNK, 128, NWIN * 32), dt.bfloat16,
                          kind="ExternalInput").ap()
    d_xk = nc.dram_tensor("xk", (NCHUNK, 128, NWIN * Mp1), dt.bfloat16,
                          kind="ExternalInput").ap()
    d_wq = nc.dram_tensor("wq", (NCHUNK, NG, 128, 128), dt.bfloat16,
                          kind="ExternalInput").ap()
    d_wk = nc.dram_tensor("wk", (NCHUNK, NG, 128, 128), dt.bfloat16,
                          kind="ExternalInput").ap()
    d_wv = nc.dram_tensor("wv", (NCHUNK, NG, 128, 128), dt.bfloat16,
                          kind="ExternalInput").ap()
    d_wp = nc.dram_tensor("wp", (NG, 4, 128, 128), dt.bfloat16,
                          kind="ExternalInput").ap()
    d_bias = nc.dram_tensor("biast", (NG, 128, Mp1), dt.float32,
                            kind="ExternalInput").ap()
    d_mask = nc.dram_tensor("maskt", (4, NG, 128, Mp1), dt.float32,
                            kind="ExternalInput").ap()
    d_bp = nc.dram_tensor("bp", (4, 128, 1), dt.float32,
                          kind="ExternalInput").ap()
    d_id = nc.dram_tensor("ident", (128, 128), dt.bfloat16,
                          kind="ExternalInput").ap()
    d_out = nc.dram_tensor("outT", (4, 128, NWIN * Np1), dt.float32,
                           kind="ExternalOutput").ap()

    @with_exitstack
    def kern(ctx: ExitStack, tc: tile.TileContext):
        nc = tc.nc
        const = ctx.enter_context(tc.tile_pool(name="const", bufs=1))
        xpool = ctx.enter_context(tc.tile_pool(name="x", bufs=2))
        fmpool = ctx.enter_context(tc.tile_pool(name="fm", bufs=2))
        atpool = ctx.enter_context(tc.tile_pool(name="at", bufs=1))
        smpool = ctx.enter_context(tc.tile_pool(name="sm", bufs=2))
        opool = ctx.enter_context(tc.tile_pool(name="o", bufs=2))
        ps_p = ctx.enter_context(tc.tile_pool(name="psp", bufs=2, space="PSUM"))
        ps_at = ctx.enter_context(tc.tile_pool(name="psat", bufs=2, space="PSUM"))
        ps_t = ctx.enter_context(tc.tile_pool(name="pst", bufs=1, space="PSUM"))
        ps_av = ctx.enter_context(tc.tile_pool(name="psav", bufs=2, space="PSUM"))

        t_wq = [[const.tile([128, 128], dt.bfloat16, tag=f"wq{c}_{a}", name=f"wq{c}_{a}")
                 for a in range(NG)] for c in range(NCHUNK)]
        t_wk = [[const.tile([128, 128], dt.bfloat16, tag=f"wk{c}_{a}", name=f"wk{c}_{a}")
                 for a in range(NG)] for c in range(NCHUNK)]
        t_wv = [[const.tile([128, 128], dt.bfloat16, tag=f"wv{c}_{a}", name=f"wv{c}_{a}")
                 for a in range(NG)] for c in range(NCHUNK)]
        t_wp = [[const.tile([128, 128], dt.bfloat16, tag=f"wp{a}_{m}", name=f"wp{a}_{m}")
                 for m in range(4)] for a in range(NG)]
        for c in range(NCHUNK):
            for a in range(NG):
                nc.sync.dma_start(t_wq[c][a][:], d_wq[c, a])
                nc.sync.dma_start(t_wk[c][a][:], d_wk[c, a])
                nc.sync.dma_start(t_wv[c][a][:], d_wv[c, a])
        for a in range(NG):
            for m in range(4):
                nc.sync.dma_start(t_wp[a][m][:], d_wp[a, m])
        t_bias = [const.tile([128, Mp1], dt.float32, tag=f"bias{a}", name=f"bias{a}")
                  for a in range(NG)]
        for a in range(NG):
            nc.sync.dma_start(t_bias[a][:], d_bias[a])
        t_mask = [[const.tile([128, Mp1], dt.float32, tag=f"mask{v}_{a}", name=f"mask{v}_{a}")
                   for a in range(NG)] for v in range(4)]
        for v in range(4):
            for a in range(NG):
                nc.sync.dma_start(t_mask[v][a][:], d_mask[v, a])
        t_bp = [const.tile([128, 1], dt.float32, tag=f"bp{m}", name=f"bpt{m}") for m in range(4)]
        for m in range(4):
            nc.sync.dma_start(t_bp[m][:], d_bp[m])
        t_id = const.tile([128, 128], dt.bfloat16, tag="ident", name="identt")
        nc.sync.dma_start(t_id[:], d_id[:])

        def mask_variant(wl):
            r = wl % NW_CONST
            if r == 0: return 0
            if r == 1: return 1
            if r == NW_CONST - 2: return 2
            if r == NW_CONST - 1: return 3
            return None

        for b in range(NBLK):
            w0 = b * WBLK
            W = WBLK
            NQ, NK = W * Np1, W * Mp1
            NQ2 = W * 32
            t_xq = [xpool.tile([128, NQ2], dt.bfloat16, tag=f"xq{c}", name=f"txq{b}_{c}")
                    for c in range(NCHUNK)]
            t_xk = [xpool.tile([128, NK], dt.bfloat16, tag=f"xk{c}", name=f"txk{b}_{c}")
                    for c in range(NCHUNK)]
            for c in range(NCHUNK):
                nc.sync.dma_start(t_xq[c][:], d_xq[c, :, w0*32:w0*32+NQ2])
                nc.sync.dma_start(t_xk[c][:], d_xk[c, :, w0*Mp1:w0*Mp1+NK])

            # --- projections ---
            t_q, t_k, t_v = [], [], []
            for kind, wmat, xin, nfree, dest in (
                ("q", t_wq, t_xq, NQ2, t_q),
                ("k", t_wk, t_xk, NK, t_k),
                ("v", t_wv, t_xk, NK, t_v),
            ):
                for a in range(NG):
                    pp = ps_p.tile([128, 512], dt.float32, tag="pp", name=f"pp{b}_{kind}{a}")[:, :nfree]
                    for c in range(NCHUNK):
                        nc.tensor.matmul(pp[:], wmat[c][a][:], xin[c][:],
                                         start=(c == 0), stop=(c == NCHUNK - 1))
                    sq = fmpool.tile([128, nfree], dt.bfloat16, tag=f"{kind}{a}", name=f"t{kind}{b}_{a}")
                    _ecopy(nc, a % 2 == 0, sq[:], pp[:])
                    dest.append(sq)

            # --- attention softmax per group ---
            t_n = []
            for a in range(NG):
                pat = ps_at.tile([128, 512], dt.float32, tag="pat", name=f"pat{b}_{a}")[:, :NK]
                for w in range(W):
                    for j in range(4):
                        nc.tensor.matmul(
                            pat[32*j:32*j+32, w*Mp1:(w+1)*Mp1],
                            t_q[a][32*j:32*j+32, w*32:(w+1)*32],
                            t_k[a][32*j:32*j+32, w*Mp1:(w+1)*Mp1],
                            start=True, stop=True,
                            tile_position=(32*j, 32*j),
                            skip_group_check=True,
                        )
                bb = t_bias[a][:].unsqueeze(1).broadcast_to((128, W, Mp1))
                pat3 = pat[:].rearrange("p (w m) -> p w m", w=W)
                nc.vector.tensor_add(pat3, pat3, bb)
                for w in range(W):
                    v = mask_variant(w0 + w)
                    if v is not None:
                        sl = pat[:, w*Mp1:(w+1)*Mp1]
                        nc.vector.tensor_add(sl, sl, t_mask[v][a][:])
                te = smpool.tile([128, NK], dt.bfloat16, tag="te", name=f"te{b}_{a}")
                nc.scalar.activation(te[:], pat[:],
                                     mybir.ActivationFunctionType.Exp)
                ts = smpool.tile([128, W], dt.float32, tag="ts", name=f"ts{b}_{a}")
                nc.vector.tensor_reduce(
                    ts[:], te[:].rearrange("p (w m) -> p w m", w=W),
                    axis=mybir.AxisListType.X, op=mybir.AluOpType.add)
                tr = smpool.tile([128, W], dt.float32, tag="tr", name=f"tr{b}_{a}")
                nc.vector.reciprocal(tr[:], ts[:])
                tn = smpool.tile([128, NK], dt.bfloat16, tag=f"tn{a}", name=f"tn{b}_{a}")
                rb = tr[:].unsqueeze(-1).broadcast_to((128, W, Mp1))
                nc.vector.tensor_mul(
                    tn[:].rearrange("p (w m) -> p w m", w=W),
                    te[:].rearrange("p (w m) -> p w m", w=W), rb)
                t_n.append(tn)

            # --- transposes: attn^T and v-nat per window ---
            t_aT, t_vT = [], []
            for w in range(W):
                taT = atpool.tile([Mp1, NG * 128], dt.bfloat16, tag=f"aT{w}", name=f"taT{b}_{w}")
                tvT = atpool.tile([Mp1, NG * 128], dt.bfloat16, tag=f"vT{w}", name=f"tvT{b}_{w}")
                for g0 in range(0, NG, 4):
                    gn = min(4, NG - g0)
                    paT = ps_t.tile([Mp1, 1024], dt.bfloat16, tag="paT", name=f"paT{b}_{w}_{g0}")[:, :512]
                    pvT = ps_t.tile([Mp1, 1024], dt.bfloat16, tag="pvT", name=f"pvT{b}_{w}_{g0}")[:, :512]
                    for gi in range(gn):
                        a = g0 + gi
                        nc.tensor.transpose(
                            paT[:, gi*128:(gi+1)*128],
                            t_n[a][:, w*Mp1:(w+1)*Mp1], t_id[:])
                        nc.tensor.transpose(
                            pvT[:, gi*128:(gi+1)*128],
                            t_v[a][:, w*Mp1:(w+1)*Mp1], t_id[:])
                    _ecopy(nc, w % 2 == 0, taT[:, g0*128:(g0+gn)*128],
                           paT[:, :gn*128])
                    _ecopy(nc, w % 2 == 1, tvT[:, g0*128:(g0+gn)*128],
                           pvT[:, :gn*128])
                t_aT.append(taT)
                t_vT.append(tvT)

            # --- AV ---
            t_ao = []
            for a in range(NG):
                pav = ps_av.tile([128, 512], dt.float32, tag="pav", name=f"pav{b}_{a}")[:, :NQ]
                for w in range(W):
                    for j in range(4):
                        cb = a * 128 + 32 * j
                        nc.tensor.matmul(
                            pav[32*j:32*j+32, w*Np1:(w+1)*Np1],
                            t_vT[w][:, cb:cb+32],
                            t_aT[w][:, cb:cb+21],
                            start=True, stop=True,
                            tile_position=(0, 32*j),
                            skip_group_check=True,
                        )
                so = fmpool.tile([128, NQ], dt.bfloat16, tag=f"ao{a}", name=f"tao{b}_{a}")
                _ecopy(nc, a % 2 == 0, so[:], pav[:])
                t_ao.append(so)

            # --- output projection ---
            for m in range(4):
                po = ps_p.tile([128, 512], dt.float32, tag="pp", name=f"po{b}_{m}")[:, :NQ]
                for a in range(NG):
                    nc.tensor.matmul(po[:], t_wp[a][m][:], t_ao[a][:],
                                     start=(a == 0), stop=(a == NG - 1))
                to = opool.tile([128, NQ], dt.float32, tag=f"to{m}", name=f"tto{b}_{m}")
                nc.vector.tensor_scalar(to[:], po[:], t_bp[m][:], None,
                                        mybir.AluOpType.add)
                nc.sync.dma_start(d_out[m, :, w0*Np1:w0*Np1+NQ], to[:])

    with tile.TileContext(nc) as tc:
        kern(tc)
    nc.compile()
    return nc


def _prep_host(x, x_, mask_left, mask_right, nW, Wq, bq, Wkv, bkv, Wproj,
               bproj, rel_table, cls_up, cls_down, cls_self):
    H, D = NUM_HEADS, HEAD_DIM
    scale = float(D) ** -0.5
    x = np.asarray(x); x_ = np.asarray(x_)
    Wq = np.asarray(Wq); bq = np.asarray(bq)
    Wkv = np.asarray(Wkv); bkv = np.asarray(bkv)
    Wproj = np.asarray(Wproj); bproj = np.asarray(bproj)
    rel_table = np.asarray(rel_table)
    cls_up = np.asarray(cls_up); cls_down = np.asarray(cls_down)
    cls_self = np.asarray(cls_self)

    maxDisparity = N_TOK - 1 + (M_TOK - N_TOK) // 2
    n_i = np.arange(N_TOK)[:, None]
    m_i = np.arange(M_TOK)[None, :]
    rel_idx = (n_i - (m_i - (M_TOK - N_TOK) // 2)) + maxDisparity
    rel_bias = np.transpose(rel_table[rel_idx], (2, 0, 1))
    top = np.concatenate([cls_self, cls_up], axis=-1)
    bot = np.concatenate([cls_down, rel_bias[None]], axis=-1)
    bias = np.concatenate([top, bot], axis=2)[0]              # (H,Np1,Mp1)

    def p32_weights(Wmat, bvec, scale_):
        out = np.zeros((NCHUNK, NG, 128, 128), dtype=np.float32)
        Wext = np.zeros((KC, 720), dtype=np.float32)
        Wext[:C] = Wmat.T * scale_
        Wext[C] = bvec * scale_
        for a in range(NG):
            for j in range(4):
                h = 4 * a + j
                blk = Wext[:, h*20:(h+1)*20]
                for c in range(NCHUNK):
                    out[c, a, :, 32*j:32*j+20] = blk[c*128:(c+1)*128]
        return out.astype(bf16)

    wq = p32_weights(Wq, bq, scale)
    wk = p32_weights(Wkv[:720], bkv[:720], 1.0)
    wv = p32_weights(Wkv[720:], bkv[720:], 1.0)

    wp = np.zeros((NG, 4, 128, 128), dtype=np.float32)
    for a in range(NG):
        for j in range(4):
            h = 4 * a + j
            for m in range(4):
                wp[a, m, 32*j:32*j+20, :100] = \
                    Wproj[m*100:(m+1)*100, h*20:(h+1)*20].T
    wp = wp.astype(bf16)

    biast = np.zeros((NG, 128, Mp1), dtype=np.float32)
    for a in range(NG):
        for j in range(4):
            biast[a, 32*j:32*j+21] = bias[4*a+j]
    mask_l = np.asarray(mask_left); mask_r = np.asarray(mask_right)
    maskt = np.zeros((4, NG, 128, Mp1), dtype=np.float32)
    for v in range(4):
        msk = mask_l[v] if v < 2 else mask_r[v - 2]
        add = np.where(msk == 1, MASK_BIG, 0.0).astype(np.float32)
        for a in range(NG):
            for j in range(4):
                maskt[v, a, 32*j:32*j+21] = add

    bp = np.zeros((4, 128, 1), dtype=np.float32)
    for m in range(4):
        bp[m, :100, 0] = bproj[m*100:(m+1)*100]
    ident = np.eye(128, dtype=np.float32).astype(bf16)

    xqs, xks = [], []
    for s in range(NCORES):
        xs = x[s*NWIN:(s+1)*NWIN].astype(np.float32)
        x_s = x_[s*NWIN:(s+1)*NWIN].astype(np.float32)
        xe = np.zeros((KC, NWIN, 32), dtype=np.float32)
        xe[:C, :, :Np1] = xs.transpose(2, 0, 1)
        xe[C, :, :Np1] = 1.0
        xe = xe.reshape(KC, NWIN * 32)
        xke = np.zeros((KC, NWIN * Mp1), dtype=np.float32)
        xke[:C] = x_s.reshape(-1, C).T
        xke[C] = 1.0
        xqs.append(np.ascontiguousarray(xe.reshape(NCHUNK, 128, -1)).astype(bf16))
        xks.append(np.ascontiguousarray(xke.reshape(NCHUNK, 128, -1)).astype(bf16))

    shared = dict(wq=wq, wk=wk, wv=wv, wp=wp, biast=biast, maskt=maskt,
                  bp=bp, ident=ident)
    return shared, xqs, xks


def kernel(**inputs) -> np.ndarray:
    if "prog" not in _cache:
        _cache["prog"] = _build_program()
    nc = _cache["prog"]
    shared, xqs, xks = _prep_host(**inputs)
    in_maps = []
    for s in range(NCORES):
        m = dict(shared)
        m["xq"] = xqs[s]
        m["xk"] = xks[s]
        in_maps.append(m)
    if SIM:
        from concourse.bass_interp import CoreSim
        outs = []
        for s in range(NCORES):
            sim = CoreSim(nc, trace=False, require_finite=False,
                          require_nnan=False)
            for k, v in in_maps[s].items():
                sim.tensor(k)[:] = v
            sim.simulate()
            outs.append(np.array(sim.tensor("outT")))
    else:
        res = bass_utils.run_bass_kernel_spmd(nc, in_maps,
                                              core_ids=list(range(NCORES)))
        outs = [res.results[s]["outT"] for s in range(NCORES)]
    pieces = []
    for s in range(NCORES):
        oT = outs[s]
        o = np.concatenate([oT[m, :100] for m in range(4)], axis=0)
        pieces.append(o.T.reshape(NWIN, Np1, C))
    return np.concatenate(pieces, axis=0).astype(np.float32)


# revision 13
# speedup vs baseline: 12.1000x; 1.0412x over previous
"""BolT windowed-attention kernel for 8 TRN2 NeuronCores (self-contained)."""
import sys, os
sys.path.insert(0, "/opt/trn_rl_repo")
import numpy as np
import ml_dtypes
from contextlib import ExitStack

from concourse import bass, bacc, tile
import concourse.mybir as mybir
from concourse import bass_utils
from concourse._compat import with_exitstack

bf16 = ml_dtypes.bfloat16

# Problem constants (hardcoded per spec)
NUM_HEADS = 36
HEAD_DIM = 20
B, NW_CONST = 32, 100
B_ = B * NW_CONST          # 3200
C = 400
N_TOK, M_TOK = 20, 40
Np1, Mp1 = 21, 41
NCORES = int(os.environ.get("BOLT_NCORES", "8"))
NWIN = int(os.environ.get("BOLT_NWIN", str(B_ // 8)))  # windows per core
WBLK = 8
NBLK = NWIN // WBLK
assert NWIN % WBLK == 0
KC = 512                   # padded contraction (400 C + 1 ones + pad)
NCHUNK = 4
NG = 9                     # 4-head groups
MASK_BIG = -30000.0
SIM = os.environ.get("BOLT_SIM", "0") == "1"

_cache = {}


def _ecopy(nc, use_vec, out, in_):
    if use_vec:
        nc.vector.tensor_copy(out, in_)
    else:
        nc.scalar.copy(out, in_)


def _build_program():
    nc = bacc.Bacc("TRN2", target_bir_lowering=False, debug=False,
                   num_devices=NCORES)
    dt = mybir.dt
    d_xq = nc.dram_tensor("xq", (NCHUNK, 128, NWIN * 32), dt.bfloat16,
                          kind="ExternalInput").ap()
    d_xk = nc.dram_tensor("xk", (NCHUNK, 128, NWIN * Mp1), dt.bfloat16,
                          kind="ExternalInput").ap()
    d_wq = nc.dram_tensor("wq", (NCHUNK, NG, 128, 128), dt.bfloat16,
                          kind="ExternalInput").ap()
    d_wk = nc.dram_tensor("wk", (NCHUNK, NG, 128, 128), dt.bfloat16,
                          kind="ExternalInput").ap()
    d_wv = nc.dram_tensor("wv", (NCHUNK, NG, 128, 128), dt.bfloat16,
                          kind="ExternalInput").ap()
    d_wp = nc.dram_tensor("wp", (NG, 4, 128, 128), dt.bfloat16,
                          kind="ExternalInput").ap()
    d_bias = nc.dram_tensor("biast", (NG, 128, Mp1), dt.float32,
                            kind="ExternalInput").ap()
    d_mask = nc.dram_tensor("maskt", (4, NG, 128, Mp1), dt.float32,
                            kind="ExternalInput").ap()
    d_bp = nc.dram_tensor("bp", (4, 128, 1), dt.float32,
                          kind="ExternalInput").ap()
    d_id = nc.dram_tensor("ident", (128, 128), dt.bfloat16,
                          kind="ExternalInput").ap()
    d_out = nc.dram_tensor("outT", (4, 128, NWIN * Np1), dt.float32,
                           kind="ExternalOutput").ap()

    @with_exitstack
    def kern(ctx: ExitStack, tc: tile.TileContext):
        nc = tc.nc
        const = ctx.enter_context(tc.tile_pool(name="const", bufs=1))
        xpool = ctx.enter_context(tc.tile_pool(name="x", bufs=2))
        fmpool = ctx.enter_context(tc.tile_pool(name="fm", bufs=2))
        atpool = ctx.enter_context(tc.tile_pool(name="at", bufs=1))
        smpool = ctx.enter_context(tc.tile_pool(name="sm", bufs=2))
        opool = ctx.enter_context(tc.tile_pool(name="o", bufs=2))
        ps_p = ctx.enter_context(tc.tile_pool(name="psp", bufs=2, space="PSUM"))
        ps_at = ctx.enter_context(tc.tile_pool(name="psat", bufs=4, space="PSUM"))
        ps_t = ctx.enter_context(tc.tile_pool(name="pst", bufs=2, space="PSUM"))

        t_wq = [[const.tile([128, 128], dt.bfloat16, tag=f"wq{c}_{a}", name=f"wq{c}_{a}")
                 for a in range(NG)] for c in range(NCHUNK)]
        t_wk = [[const.tile([128, 128], dt.bfloat16, tag=f"wk{c}_{a}", name=f"wk{c}_{a}")
                 for a in range(NG)] for c in range(NCHUNK)]
        t_wv = [[const.tile([128, 128], dt.bfloat16, tag=f"wv{c}_{a}", name=f"wv{c}_{a}")
                 for a in range(NG)] for c in range(NCHUNK)]
        t_wp = [[const.tile([128, 128], dt.bfloat16, tag=f"wp{a}_{m}", name=f"wp{a}_{m}")
                 for m in range(4)] for a in range(NG)]
        for c in range(NCHUNK):
            for a in range(NG):
                nc.sync.dma_start(t_wq[c][a][:], d_wq[c, a])
                nc.sync.dma_start(t_wk[c][a][:], d_wk[c, a])
                nc.sync.dma_start(t_wv[c][a][:], d_wv[c, a])
        for a in range(NG):
            for m in range(4):
                nc.sync.dma_start(t_wp[a][m][:], d_wp[a, m])
        t_bias = [const.tile([128, Mp1], dt.float32, tag=f"bias{a}", name=f"bias{a}")
                  for a in range(NG)]
        for a in range(NG):
            nc.sync.dma_start(t_bias[a][:], d_bias[a])
        t_mask = [[const.tile([128, Mp1], dt.float32, tag=f"mask{v}_{a}", name=f"mask{v}_{a}")
                   for a in range(NG)] for v in range(4)]
        for v in range(4):
            for a in range(NG):
                nc.sync.dma_start(t_mask[v][a][:], d_mask[v, a])
        t_bp = [const.tile([128, 1], dt.float32, tag=f"bp{m}", name=f"bpt{m}") for m in range(4)]
        for m in range(4):
            nc.sync.dma_start(t_bp[m][:], d_bp[m])
        t_id = const.tile([128, 128], dt.bfloat16, tag="ident", name="identt")
        nc.sync.dma_start(t_id[:], d_id[:])

        def mask_variant(wl):
            r = wl % NW_CONST
            if r == 0: return 0
            if r == 1: return 1
            if r == NW_CONST - 2: return 2
            if r == NW_CONST - 1: return 3
            return None

        for b in range(NBLK):
            w0 = b * WBLK
            W = WBLK
            NQ, NK = W * Np1, W * Mp1
            NQ2 = W * 32
            t_xq = [xpool.tile([128, NQ2], dt.bfloat16, tag=f"xq{c}", name=f"txq{b}_{c}")
                    for c in range(NCHUNK)]
            t_xk = [xpool.tile([128, NK], dt.bfloat16, tag=f"xk{c}", name=f"txk{b}_{c}")
                    for c in range(NCHUNK)]
            for c in range(NCHUNK):
                nc.sync.dma_start(t_xq[c][:], d_xq[c, :, w0*32:w0*32+NQ2])
                nc.sync.dma_start(t_xk[c][:], d_xk[c, :, w0*Mp1:w0*Mp1+NK])

            # --- projections ---
            t_q, t_k, t_v = [], [], []
            for kind, wmat, xin, nfree, dest in (
                ("q", t_wq, t_xq, NQ2, t_q),
                ("k", t_wk, t_xk, NK, t_k),
                ("v", t_wv, t_xk, NK, t_v),
            ):
                for a in range(NG):
                    pp = ps_p.tile([128, 512], dt.float32, tag="pp", name=f"pp{b}_{kind}{a}")[:, :nfree]
                    for c in range(NCHUNK):
                        nc.tensor.matmul(pp[:], wmat[c][a][:], xin[c][:],
                                         start=(c == 0), stop=(c == NCHUNK - 1))
                    sq = fmpool.tile([128, nfree], dt.bfloat16, tag=f"{kind}{a}", name=f"t{kind}{b}_{a}")
                    _ecopy(nc, a % 2 == 0, sq[:], pp[:])
                    dest.append(sq)

            # --- attention softmax per group ---
            t_n = []
            for a in range(NG):
                pat = ps_at.tile([128, 512], dt.float32, tag="pat", name=f"pat{b}_{a}")[:, :NK]
                for w in range(W):
                    for j in range(4):
                        nc.tensor.matmul(
                            pat[32*j:32*j+32, w*Mp1:(w+1)*Mp1],
                            t_q[a][32*j:32*j+32, w*32:(w+1)*32],
                            t_k[a][32*j:32*j+32, w*Mp1:(w+1)*Mp1],
                            start=True, stop=True,
                            tile_position=(32*j, 32*j),
                            skip_group_check=True,
                        )
                bb = t_bias[a][:].unsqueeze(1).broadcast_to((128, W, Mp1))
                pat3 = pat[:].rearrange("p (w m) -> p w m", w=W)
                nc.vector.tensor_add(pat3, pat3, bb)
                for w in range(W):
                    v = mask_variant(w0 + w)
                    if v is not None:
                        sl = pat[:, w*Mp1:(w+1)*Mp1]
                        nc.vector.tensor_add(sl, sl, t_mask[v][a][:])
                te = smpool.tile([128, NK], dt.bfloat16, tag="te", name=f"te{b}_{a}")
                nc.scalar.activation(te[:], pat[:],
                                     mybir.ActivationFunctionType.Exp)
                ts = smpool.tile([128, W], dt.float32, tag="ts", name=f"ts{b}_{a}")
                nc.vector.tensor_reduce(
                    ts[:], te[:].rearrange("p (w m) -> p w m", w=W),
                    axis=mybir.AxisListType.X, op=mybir.AluOpType.add)
                tr = smpool.tile([128, W], dt.float32, tag="tr", name=f"tr{b}_{a}")
                nc.vector.reciprocal(tr[:], ts[:])
                tn = smpool.tile([128, NK], dt.bfloat16, tag=f"tn{a}", name=f"tn{b}_{a}")
                rb = tr[:].unsqueeze(-1).broadcast_to((128, W, Mp1))
                nc.vector.tensor_mul(
                    tn[:].rearrange("p (w m) -> p w m", w=W),
                    te[:].rearrange("p (w m) -> p w m", w=W), rb)
                t_n.append(tn)

            # --- transposes: attn^T and v-nat per window ---
            t_aT, t_vT = [], []
            for w in range(W):
                taT = atpool.tile([Mp1, NG * 128], dt.bfloat16, tag=f"aT{w}", name=f"taT{b}_{w}")
                tvT = atpool.tile([Mp1, NG * 128], dt.bfloat16, tag=f"vT{w}", name=f"tvT{b}_{w}")
                for g0 in range(0, NG, 4):
                    gn = min(4, NG - g0)
                    paT = ps_t.tile([Mp1, 1024], dt.bfloat16, tag="paT", name=f"paT{b}_{w}_{g0}")[:, :512]
                    pvT = ps_t.tile([Mp1, 1024], dt.bfloat16, tag="paT", name=f"pvT{b}_{w}_{g0}")[:, :512]
                    for gi in range(gn):
                        a = g0 + gi
                        nc.tensor.transpose(
                            paT[:, gi*128:(gi+1)*128],
                            t_n[a][:, w*Mp1:(w+1)*Mp1], t_id[:])
                        nc.tensor.transpose(
                            pvT[:, gi*128:(gi+1)*128],
                            t_v[a][:, w*Mp1:(w+1)*Mp1], t_id[:])
                    _ecopy(nc, w % 2 == 0, taT[:, g0*128:(g0+gn)*128],
                           paT[:, :gn*128])
                    _ecopy(nc, w % 2 == 1, tvT[:, g0*128:(g0+gn)*128],
                           pvT[:, :gn*128])
                t_aT.append(taT)
                t_vT.append(tvT)

            # --- AV ---
            t_ao = []
            for a in range(NG):
                pav = ps_at.tile([128, 512], dt.float32, tag="pat", name=f"pav{b}_{a}")[:, :NQ]
                for w in range(W):
                    for j in range(4):
                        cb = a * 128 + 32 * j
                        nc.tensor.matmul(
                            pav[32*j:32*j+32, w*Np1:(w+1)*Np1],
                            t_vT[w][:, cb:cb+32],
                            t_aT[w][:, cb:cb+21],
                            start=True, stop=True,
                            tile_position=(0, 32*j),
                            skip_group_check=True,
                        )
                so = fmpool.tile([128, NQ], dt.bfloat16, tag=f"ao{a}", name=f"tao{b}_{a}")
                _ecopy(nc, a % 2 == 0, so[:], pav[:])
                t_ao.append(so)

            # --- output projection ---
            for m in range(4):
                po = ps_p.tile([128, 512], dt.float32, tag="pp", name=f"po{b}_{m}")[:, :NQ]
                for a in range(NG):
                    nc.tensor.matmul(po[:], t_wp[a][m][:], t_ao[a][:],
                                     start=(a == 0), stop=(a == NG - 1))
                to = opool.tile([128, NQ], dt.float32, tag=f"to{m}", name=f"tto{b}_{m}")
                nc.vector.tensor_scalar(to[:], po[:], t_bp[m][:], None,
                                        mybir.AluOpType.add)
                nc.sync.dma_start(d_out[m, :, w0*Np1:w0*Np1+NQ], to[:])

    with tile.TileContext(nc) as tc:
        kern(tc)
    nc.compile()
    return nc


def _prep_host(x, x_, mask_left, mask_right, nW, Wq, bq, Wkv, bkv, Wproj,
               bproj, rel_table, cls_up, cls_down, cls_self):
    H, D = NUM_HEADS, HEAD_DIM
    scale = float(D) ** -0.5
    x = np.asarray(x); x_ = np.asarray(x_)
    Wq = np.asarray(Wq); bq = np.asarray(bq)
    Wkv = np.asarray(Wkv); bkv = np.asarray(bkv)
    Wproj = np.asarray(Wproj); bproj = np.asarray(bproj)
    rel_table = np.asarray(rel_table)
    cls_up = np.asarray(cls_up); cls_down = np.asarray(cls_down)
    cls_self = np.asarray(cls_self)

    maxDisparity = N_TOK - 1 + (M_TOK - N_TOK) // 2
    n_i = np.arange(N_TOK)[:, None]
    m_i = np.arange(M_TOK)[None, :]
    rel_idx = (n_i - (m_i - (M_TOK - N_TOK) // 2)) + maxDisparity
    rel_bias = np.transpose(rel_table[rel_idx], (2, 0, 1))
    top = np.concatenate([cls_self, cls_up], axis=-1)
    bot = np.concatenate([cls_down, rel_bias[None]], axis=-1)
    bias = np.concatenate([top, bot], axis=2)[0]              # (H,Np1,Mp1)

    def p32_weights(Wmat, bvec, scale_):
        out = np.zeros((NCHUNK, NG, 128, 128), dtype=np.float32)
        Wext = np.zeros((KC, 720), dtype=np.float32)
        Wext[:C] = Wmat.T * scale_
        Wext[C] = bvec * scale_
        for a in range(NG):
            for j in range(4):
                h = 4 * a + j
                blk = Wext[:, h*20:(h+1)*20]
                for c in range(NCHUNK):
                    out[c, a, :, 32*j:32*j+20] = blk[c*128:(c+1)*128]
        return out.astype(bf16)

    wq = p32_weights(Wq, bq, scale)
    wk = p32_weights(Wkv[:720], bkv[:720], 1.0)
    wv = p32_weights(Wkv[720:], bkv[720:], 1.0)

    wp = np.zeros((NG, 4, 128, 128), dtype=np.float32)
    for a in range(NG):
        for j in range(4):
            h = 4 * a + j
            for m in range(4):
                wp[a, m, 32*j:32*j+20, :100] = \
                    Wproj[m*100:(m+1)*100, h*20:(h+1)*20].T
    wp = wp.astype(bf16)

    biast = np.zeros((NG, 128, Mp1), dtype=np.float32)
    for a in range(NG):
        for j in range(4):
            biast[a, 32*j:32*j+21] = bias[4*a+j]
    mask_l = np.asarray(mask_left); mask_r = np.asarray(mask_right)
    maskt = np.zeros((4, NG, 128, Mp1), dtype=np.float32)
    for v in range(4):
        msk = mask_l[v] if v < 2 else mask_r[v - 2]
        add = np.where(msk == 1, MASK_BIG, 0.0).astype(np.float32)
        for a in range(NG):
            for j in range(4):
                maskt[v, a, 32*j:32*j+21] = add

    bp = np.zeros((4, 128, 1), dtype=np.float32)
    for m in range(4):
        bp[m, :100, 0] = bproj[m*100:(m+1)*100]
    ident = np.eye(128, dtype=np.float32).astype(bf16)

    xqs, xks = [], []
    for s in range(NCORES):
        xs = x[s*NWIN:(s+1)*NWIN].astype(np.float32)
        x_s = x_[s*NWIN:(s+1)*NWIN].astype(np.float32)
        xe = np.zeros((KC, NWIN, 32), dtype=np.float32)
        xe[:C, :, :Np1] = xs.transpose(2, 0, 1)
        xe[C, :, :Np1] = 1.0
        xe = xe.reshape(KC, NWIN * 32)
        xke = np.zeros((KC, NWIN * Mp1), dtype=np.float32)
        xke[:C] = x_s.reshape(-1, C).T
        xke[C] = 1.0
        xqs.append(np.ascontiguousarray(xe.reshape(NCHUNK, 128, -1)).astype(bf16))
        xks.append(np.ascontiguousarray(xke.reshape(NCHUNK, 128, -1)).astype(bf16))

    shared = dict(wq=wq, wk=wk, wv=wv, wp=wp, biast=biast, maskt=maskt,
                  bp=bp, ident=ident)
    return shared, xqs, xks


def kernel(**inputs) -> np.ndarray:
    if "prog" not in _cache:
        _cache["prog"] = _build_program()
    nc = _cache["prog"]
    shared, xqs, xks = _prep_host(**inputs)
    in_maps = []
    for s in range(NCORES):
        m = dict(shared)
        m["xq"] = xqs[s]
        m["xk"] = xks[s]
        in_maps.append(m)
    if SIM:
        from concourse.bass_interp import CoreSim
        outs = []
        for s in range(NCORES):
            sim = CoreSim(nc, trace=False, require_finite=False,
                          require_nnan=False)
            for k, v in in_maps[s].items():
                sim.tensor(k)[:] = v
            sim.simulate()
            outs.append(np.array(sim.tensor("outT")))
    else:
        res = bass_utils.run_bass_kernel_spmd(nc, in_maps,
                                              core_ids=list(range(NCORES)))
        outs = [res.results[s]["outT"] for s in range(NCORES)]
    pieces = []
    for s in range(NCORES):
        oT = outs[s]
        o = np.concatenate([oT[m, :100] for m in range(4)], axis=0)
        pieces.append(o.T.reshape(NWIN, Np1, C))
    return np.concatenate(pieces, axis=0).astype(np.float32)
